# revision 32
# baseline (speedup 1.0000x reference)
"""Trainium2 Bass kernel for nn_DisjointSTModel (GNN message passing + GRU).

Algorithm refactoring (validated vs reference in numpy):
  - A_hat = S @ A_raw @ S with S = diag(1/sqrt(max(deg,1))): normalization is
    separable, so SpMM hops become pure gather-sums with per-node scaling.
  - ELL gather-sum: nodes degree-sorted, chunked; each chunk processed as
    D_max levels of dma_gather (single-packet, <=1024 idxs) + DVE adds.
  - Layer 2 in Horner form: out2 = P0 + S A_raw [S P1 + S^2 A_raw (S P2)],
    with Pk = Hin @ W2_k computed on the tensor engine once (bf16 operands).
  - ALL hops are node-sharded over the 8 cores.  Layer-2 hop sources are the
    full-width [NP, 16*64] matrices in fp8-e4m3 (x64 pre-scale folded into
    the dinv factors): 1KB gather rows, ~27 gathers of 1024 idxs per core
    per hop, with the ELL level-sums done on the tensor engine as fp8
    identity matmuls accumulating in PSUM (exact: x1.0 weights, f32 banks).
  - Layer-1 hops (64-f32 = 256B rows) spread their gathers over 2 SWDGE
    queues so descriptor generation parallelizes across q7 pairs.
  - GRU stacks [h|x] on 128 partitions (2 bf16 matmuls per chunk: r|z fused
    by contraction, nx|nh in one psum) and computes gates with Tanh
    activations read straight from PSUM (sigmoid as scaled tanh, so the
    activation table never swaps).

Launches: A0 (hop1) -> AS (hop2 + Hin + P0/P1/P2 matmuls) ->
H1 (hop3 + V) -> H2 (hop4 + tail) -> GB (GRU + head).  Host work between
launches is only slicing/concatenation (cross-core exchange).
"""
import numpy as np
import ml_dtypes

import concourse.bacc as bacc
import concourse.mybir as mybir
import concourse.tile as tile
from concourse.bass_utils import run_bass_kernel_spmd
from concourse.library_config import mlp

P = 128
N = 10000
B = 2
T = 8
M = B * T          # 16 (b,t) columns
HID = 64
FW = M * HID       # 1024 full row width
NP = 10240         # padded node count (80 cols of 128)
NCOL = NP // P     # 80
NCORES = 8
ZERO_ROW = NP - 1
LPC = NP // NCORES // P   # local cols per core for node-sharded phases = 10
LROWS = NP // NCORES      # 1280 local nodes per core

NCHUNK = 128       # node-sharded ELL chunk (local positions)
NLEV = 8           # levels per gather (num_idxs = 1024)

F32 = mybir.dt.float32
BF16 = mybir.dt.bfloat16
I16 = mybir.dt.int16
F8 = mybir.dt.float8e4
ADD = mybir.AluOpType.add
MULT = mybir.AluOpType.mult
AFT = mybir.ActivationFunctionType
NPBF16 = ml_dtypes.bfloat16
NPF8 = ml_dtypes.float8_e4m3
SC = 64.0          # fp8 pre-scale for the layer-2 hop sources

_CACHE = {}
TIMING_REPS = 1      # >1: wrap each launch body in a repeat loop (for timing)
LAST_WALLS = {}      # launch name -> wall seconds of the spmd call



# ----------------------------------------------------------------------------
# host-side graph preprocessing
# ----------------------------------------------------------------------------

def _pack_idxs(idx_flat):
    t = idx_flat.astype(np.int16).reshape(-1, 16).T
    return np.ascontiguousarray(np.tile(t, (8, 1)))


def _ell_table(deg, offs, src_sorted, node_of, chunk, maxlev, forced_levels=None):
    """Build an ELL gather table for output rows node_of[0..len), chunked by
    `chunk` rows.  Returns (idx_flat, gathers, levels_per_chunk).
    gathers: list of (chunk_j, col16, num_idxs, lv).
    Padding slots point at randomized pad rows (N..NP-1, all zeroed) rather
    than a single zero row, to avoid an HBM hot-spot."""
    n = len(node_of)
    assert n % chunk == 0
    nchunk = n // chunk
    rng = np.random.default_rng(1234)
    idx_parts, gathers, levels = [], [], []
    col = 0
    for j in range(nchunk):
        nodes = node_of[j * chunk:(j + 1) * chunk]
        dj = int(deg[nodes].max())
        if forced_levels is not None:
            dj = forced_levels[j]
        dj = max(dj, 1)
        levels.append(dj)
        lvl = np.arange(dj)[:, None]
        pos = offs[nodes][None, :] + lvl
        valid = lvl < deg[nodes][None, :]
        pad = rng.integers(N, NP, size=pos.shape)
        slot = np.where(valid, src_sorted[np.minimum(pos, len(src_sorted) - 1)],
                        pad)
        for a in range(0, dj, maxlev):
            lv = min(maxlev, dj - a)
            idx_parts.append(slot[a:a + lv].reshape(-1))
            gathers.append((j, col, lv * chunk, lv))
            col += lv * chunk // 16
    return np.concatenate(idx_parts), gathers, levels


def _prep(edge_src, edge_dst):
    deg_f = np.zeros(N, np.float32)
    np.add.at(deg_f, edge_dst, 1.0)
    dinv = (1.0 / np.sqrt(np.maximum(deg_f, 1.0))).astype(np.float32)

    order = np.argsort(-deg_f, kind="stable")     # old ids in new-rank order
    perm = np.empty(N, np.int64)
    perm[order] = np.arange(N)

    src_n = perm[edge_src]
    dst_n = perm[edge_dst]
    deg_n = np.zeros(NP, np.int64)
    np.add.at(deg_n, dst_n, 1)
    dinv_n = np.ones(NP, np.float32)
    dinv_n[:N] = dinv[order]

    o = np.argsort(dst_n, kind="stable")
    src_sorted = src_n[o].astype(np.int64)
    offs = np.zeros(NP + 1, np.int64)
    np.cumsum(deg_n, out=offs[1:])

    # node-sharded tables: rank i -> core i%8, local pos i//8.
    # levels forced to the max over cores so all 8 cores share one program.
    per_core_nodes = [np.arange(NP)[c::NCORES] for c in range(NCORES)]
    nlev_chunks = NP // NCORES // NCHUNK
    forced = []
    for j in range(nlev_chunks):
        forced.append(max(
            max(int(deg_n[per_core_nodes[c][j * NCHUNK:(j + 1) * NCHUNK]].max()), 1)
            for c in range(NCORES)))
    nidx, ngath = [], None
    for c in range(NCORES):
        fi, fg, _ = _ell_table(deg_n, offs, src_sorted, per_core_nodes[c],
                               NCHUNK, NLEV, forced_levels=forced)
        nidx.append(_pack_idxs(fi))
        ngath = fg

    return dict(order=order, perm=perm, dinv_n=dinv_n,
                nidx=nidx, ngath=ngath,
                node_of=per_core_nodes)


# ----------------------------------------------------------------------------
# device programs
# ----------------------------------------------------------------------------

def _wrap(v):
    """[NP] -> [128, NCOL] node-major wrap (n = col*128 + p)."""
    return np.ascontiguousarray(v.reshape(NCOL, P).T)


def _hop_gathers(nc, gbuf, src_d, idx_sb, gathers, acc, elem, dt, queues=1):
    """Emit gathers + accumulating adds: acc[:, j, :] += sum of levels.
    Multi-level gathers fold their levels with one strided DVE reduce."""
    for i, (j, c0, nidx, lv) in enumerate(gathers):
        G = gbuf.tile([P, NLEV, elem], dt, tag="G")
        g = G[:, : nidx // P, :]
        nc.gpsimd.dma_gather(g, src_d[:], idx_sb[:, c0:c0 + nidx // 16],
                             nidx, nidx, elem, single_packet=True,
                             queue_num=i % queues)
        a = acc[:, j:j + 1, :]
        if lv == 1:
            nc.vector.tensor_tensor(out=a, in0=a, in1=g[:, 0:1, :], op=ADD)
        else:
            t = gbuf.tile([P, 1, elem], F32, tag="R")
            nc.vector.tensor_reduce(
                out=t[:, 0, :], in_=g.rearrange("p l f -> p f l"),
                axis=mybir.AxisListType.X, op=ADD)
            nc.vector.tensor_tensor(out=a, in0=a, in1=t[:], op=ADD)


def _build_hop_node(tables, first, reps=1):
    """A0 (first=True): xs = x*dinv; hop1 -> z1t_s, z1s_s.
       A1 (first=False): hop2 over z1s -> z2t_s."""
    nc = bacc.Bacc(None, target_bir_lowering=False, num_swdge_queues=2)
    idx_cols = tables["nidx"][0].shape[1]
    idx_d = nc.dram_tensor("idxn", [P, idx_cols], I16, kind="ExternalInput")
    dinv_d = nc.dram_tensor("dinvl", [P, LPC], F32, kind="ExternalInput")
    dinv2_d = nc.dram_tensor("dinv2l", [P, LPC], F32, kind="ExternalInput")

    if first:
        x_d = nc.dram_tensor("x16", [NP, 16], F32, kind="ExternalInput")
        dinvw_d = nc.dram_tensor("dinvw", [P, NCOL], F32, kind="ExternalInput")
        zt_d = nc.dram_tensor("z1t_s", [LROWS, 16], F32, kind="ExternalOutput")
        zs_d = nc.dram_tensor("z1s_s", [LROWS, 64], F32, kind="ExternalOutput")
    else:
        src_in = nc.dram_tensor("z1s", [NP, 64], F32, kind="ExternalInput")
        zt_d = nc.dram_tensor("z2t_s", [LROWS, 16], F32, kind="ExternalOutput")

    with tile.TileContext(nc) as tc:
        with (
            tc.tile_pool(name="pool", bufs=1) as pool,
            tc.tile_pool(name="gbuf", bufs=4) as gbuf,
            tc.tile_pool(name="dram", bufs=1, space="DRAM") as dram,
        ):
            nc.gpsimd.load_library(mlp)
            idx_sb = pool.tile([P, idx_cols], I16)
            nc.sync.dma_start(idx_sb[:], idx_d[:])
            dinvl = pool.tile([P, LPC], F32)
            nc.sync.dma_start(dinvl[:], dinv_d[:])

            if first:
                dinvw = pool.tile([P, NCOL], F32)
                nc.sync.dma_start(dinvw[:], dinvw_d[:])
                xw = pool.tile([P, NCOL, 16], F32)
                nc.sync.dma_start(xw[:], x_d.rearrange("(c p) f -> p c f", p=P))
                dinv2l = pool.tile([P, LPC], F32)
                nc.sync.dma_start(dinv2l[:], dinv2_d[:])
                src_h = dram.tile([NP, 64], F32)

            def body(_=None):
                if first:
                    # xs = x * dinv (full, replicated on every core), pad to 64
                    xs = pool.tile([P, NCOL, 64], F32, tag="xs")
                    nc.vector.memset(xs[:], 0.0)
                    nc.vector.tensor_tensor(
                        out=xs[:, :, 0:16], in0=xw[:],
                        in1=dinvw[:, :, None].to_broadcast([P, NCOL, 16]), op=MULT)
                    nc.sync.dma_start(src_h[:].rearrange("(c p) f -> p c f", p=P), xs[:])
                    src_ap = src_h
                else:
                    src_ap = src_in

                acc = pool.tile([P, LPC, 64], F32, tag="acc")
                nc.vector.memset(acc[:], 0.0)
                _hop_gathers(nc, gbuf, src_ap, idx_sb, tables["ngath"],
                             acc, 64, F32, queues=2)

                # z*t = acc[:, :, :16] * dinv_local  -> [1280, 16]
                zt = pool.tile([P, LPC, 16], F32, tag="zt")
                nc.vector.tensor_tensor(
                    out=zt[:], in0=acc[:, :, 0:16],
                    in1=dinvl[:, :, None].to_broadcast([P, LPC, 16]), op=MULT)
                nc.sync.dma_start(zt_d.rearrange("(c p) f -> p c f", p=P), zt[:])

                if first:
                    zs = pool.tile([P, LPC, 64], F32, tag="zs")
                    nc.vector.tensor_tensor(
                        out=zs[:], in0=acc[:],
                        in1=dinv2l[:, :, None].to_broadcast([P, LPC, 64]), op=MULT)
                    nc.sync.dma_start(zs_d.rearrange("(c p) f -> p c f", p=P), zs[:])

            if reps == 1:
                body()
            else:
                with tc.For_i(0, reps, 1):
                    body()
    nc.compile()
    return nc


def _build_as(tables, reps=1):
    """AS: hop2 (z2 = dinv * A_raw @ z1s, node-sharded gathers), then
    Hin = relu(x W1_0 + z1 W1_1 + z2 W1_2 + b1) for the core's local nodes
    (all 16 columns); P0/P1'/W = the three W2 matmuls, bf16 out.
    P1' = dinv*P1 and W = dinv*P2 (dinv folded in here)."""
    nc = bacc.Bacc(None, target_bir_lowering=False, num_swdge_queues=2)
    idx_cols = tables["nidx"][0].shape[1]
    idx_d = nc.dram_tensor("idxn", [P, idx_cols], I16, kind="ExternalInput")
    src_in = nc.dram_tensor("z1s", [NP, 64], F32, kind="ExternalInput")
    zx_d = nc.dram_tensor("zx", [LROWS, 16], F32, kind="ExternalInput")
    z1_d = nc.dram_tensor("z1", [LROWS, 16], F32, kind="ExternalInput")
    dinv_d = nc.dram_tensor("dinvl", [P, LPC], F32, kind="ExternalInput")
    dinvsc_d = nc.dram_tensor("dinvsc", [P, LPC], F32, kind="ExternalInput")
    w1_d = nc.dram_tensor("w1", [3, HID], F32, kind="ExternalInput")
    b1_d = nc.dram_tensor("b1", [1, HID], F32, kind="ExternalInput")
    w2_d = nc.dram_tensor("w2all", [P, 384], BF16, kind="ExternalInput")
    w_d = nc.dram_tensor("w", [LROWS, FW], F8, kind="ExternalOutput")
    p1_d = nc.dram_tensor("p1", [LROWS, FW], F8, kind="ExternalOutput")
    p0_d = nc.dram_tensor("p0", [LROWS, FW], BF16, kind="ExternalOutput")

    from concourse.masks import make_identity
    with tile.TileContext(nc) as tc:
        with (
            tc.tile_pool(name="pool", bufs=1) as pool,
            tc.tile_pool(name="gbuf", bufs=4) as gbuf,
            tc.tile_pool(name="psA", bufs=3, space="PSUM") as psA,
            tc.tile_pool(name="psT", bufs=3, space="PSUM") as psT,
        ):
            nc.gpsimd.load_library(mlp)
            idx_sb = pool.tile([P, idx_cols], I16)
            nc.sync.dma_start(idx_sb[:], idx_d[:])
            zx = pool.tile([P, LPC, 16], F32)
            nc.sync.dma_start(zx[:], zx_d.rearrange("(c p) m -> p c m", p=P))
            z1 = pool.tile([P, LPC, 16], F32)
            nc.sync.dma_start(z1[:], z1_d.rearrange("(c p) m -> p c m", p=P))
            dinvl = pool.tile([P, LPC], F32)
            nc.sync.dma_start(dinvl[:], dinv_d[:])
            dinvsc = pool.tile([P, LPC], F32)
            nc.sync.dma_start(dinvsc[:], dinvsc_d[:])
            w1rep = pool.tile([P, 3, HID], F32)
            for k in range(3):
                nc.sync.dma_start(w1rep[:, k, :], w1_d[k:k + 1, :].to_broadcast([P, HID]))
            b1rep = pool.tile([P, HID], F32)
            nc.sync.dma_start(b1rep[:], b1_d[:].to_broadcast([P, HID]))
            w2all = pool.tile([P, 384], BF16)
            nc.sync.dma_start(w2all[:], w2_d[:])
            ident = pool.tile([P, P], F32)
            make_identity(nc, ident[:])

            def bc4(t):
                return t[:, None, None, :].to_broadcast([P, LPC, 16, HID])

            def body(_=None):
                # ---- hop2: z2 = dinv * (A_raw @ z1s) for local nodes ----
                acc = pool.tile([P, LPC, 64], F32, tag="acc")
                nc.vector.memset(acc[:], 0.0)
                _hop_gathers(nc, gbuf, src_in, idx_sb, tables["ngath"],
                             acc, 64, F32, queues=2)
                z2 = pool.tile([P, LPC, 16], F32, tag="z2")
                nc.vector.tensor_tensor(
                    out=z2[:], in0=acc[:, :, 0:16],
                    in1=dinvl[:, :, None].to_broadcast([P, LPC, 16]), op=MULT)

                # ---- Hin (f32 accumulate, relu in place) ----
                hinf = pool.tile([P, LPC, 16, HID], F32, tag="hinf")
                tmp = pool.tile([P, LPC, 16, HID], F32, tag="tmp")
                nc.vector.tensor_tensor(
                    out=hinf[:], in0=zx[:, :, :, None].to_broadcast([P, LPC, 16, HID]),
                    in1=bc4(w1rep[:, 0, :]), op=MULT)
                for k, zk in ((1, z1), (2, z2)):
                    nc.vector.tensor_tensor(
                        out=tmp[:], in0=zk[:, :, :, None].to_broadcast([P, LPC, 16, HID]),
                        in1=bc4(w1rep[:, k, :]), op=MULT)
                    nc.vector.tensor_tensor(out=hinf[:], in0=hinf[:], in1=tmp[:], op=ADD)
                nc.vector.tensor_tensor(out=hinf[:], in0=hinf[:], in1=bc4(b1rep), op=ADD)
                nc.vector.tensor_relu(out=hinf[:], in_=hinf[:])

                # ---- transpose 128x128 tiles: [n, (m2 f)] -> [(m2 f), n] ----
                hv = hinf[:].rearrange("p c (pr m2) f -> p c pr (m2 f)", m2=2)
                hT = pool.tile([P, LPC, 8, P], BF16, tag="hT")
                for c in range(LPC):
                    for pr in range(8):
                        pt = psT.tile([P, P], F32, tag="pT")
                        nc.tensor.transpose(pt[:], hv[:, c, pr, :], ident[:])
                        nc.vector.tensor_copy(out=hT[:, c, pr, :], in_=pt[:])

                # ---- P0 | P1' | W via one [128,384] matmul per (c, pr) ----
                # W/P1' leave in fp8 scaled by SC*dinv (act engine casts).
                wsb = pool.tile([P, LPC, 8, 2, HID], F8, tag="wsb")
                p1sb = pool.tile([P, LPC, 8, 2, HID], F8, tag="p1sb")
                p0sb = pool.tile([P, LPC, 8, 2, HID], BF16, tag="p0sb")
                for c in range(LPC):
                    for pr in range(8):
                        ps = psA.tile([P, 384], F32, tag="psA")
                        nc.tensor.matmul(ps[:], hT[:, c, pr, :], w2all[:],
                                         start=True, stop=True)
                        psv = ps[:].rearrange("p (m k o) -> p m k o", m=2, k=3)
                        nc.vector.tensor_copy(out=p0sb[:, c, pr, :, :],
                                              in_=psv[:, :, 0, :])
                        nc.scalar.activation(
                            p1sb[:, c, pr, :, :], psv[:, :, 1, :], AFT.Copy,
                            scale=dinvsc[:, c:c + 1])
                        nc.scalar.activation(
                            wsb[:, c, pr, :, :], psv[:, :, 2, :], AFT.Copy,
                            scale=dinvsc[:, c:c + 1])

                nc.sync.dma_start(w_d.rearrange("(c p) f -> p c f", p=P),
                                  wsb[:].rearrange("p c pr m o -> p c (pr m o)"))
                nc.sync.dma_start(p1_d.rearrange("(c p) f -> p c f", p=P),
                                  p1sb[:].rearrange("p c pr m o -> p c (pr m o)"))
                nc.sync.dma_start(p0_d.rearrange("(c p) f -> p c f", p=P),
                                  p0sb[:].rearrange("p c pr m o -> p c (pr m o)"))

            if reps == 1:
                body()
            else:
                with tc.For_i(0, reps, 1):
                    body()
    nc.compile()
    return nc


def _build_hop_full(tables, second, reps=1):
    """H1 (second=False): r = A_raw @ W (1KB fp8 rows, SC-scaled);
       V = P1' + dinv^2*r, fp8 out.
       H2 (second=True):  s = A_raw @ V; h2 = relu((dinv/SC)*s + P0 + b2).
    Accumulation happens on the tensor engine: fp8 identity matmuls sum the
    ELL levels of each 128-node chunk straight into a PSUM bank."""
    nc = bacc.Bacc(None, target_bir_lowering=False, num_swdge_queues=2)
    idx_cols = tables["nidx"][0].shape[1]
    idx_d = nc.dram_tensor("idxn", [P, idx_cols], I16, kind="ExternalInput")
    src_d = nc.dram_tensor("src", [NP, FW], F8, kind="ExternalInput")
    id_d = nc.dram_tensor("idf8", [P, P], F8, kind="ExternalInput")
    dinv_d = nc.dram_tensor("dinvl", [P, LPC], F32, kind="ExternalInput")
    if not second:
        px_d = nc.dram_tensor("px", [LROWS, FW], F8, kind="ExternalInput")
        out_d = nc.dram_tensor("v", [LROWS, FW], F8, kind="ExternalOutput")
    else:
        px_d = nc.dram_tensor("px", [LROWS, FW], BF16, kind="ExternalInput")
        b2_d = nc.dram_tensor("b2", [1, HID], F32, kind="ExternalInput")
        out_d = nc.dram_tensor("h2", [LROWS, FW], BF16, kind="ExternalOutput")

    with tile.TileContext(nc) as tc:
        with (
            tc.tile_pool(name="pool", bufs=1) as pool,
            tc.tile_pool(name="gbuf", bufs=4) as gbuf,
            tc.tile_pool(name="tb", bufs=3) as tb,
            tc.tile_pool(name="psA", bufs=3, space="PSUM") as psA,
        ):
            nc.gpsimd.load_library(mlp)
            idx_sb = pool.tile([P, idx_cols], I16)
            nc.sync.dma_start(idx_sb[:], idx_d[:])
            idf8 = pool.tile([P, P], F8)
            nc.sync.dma_start(idf8[:], id_d[:])
            # dinvl carries dinv^2 (H1) or dinv/SC (H2) from the host
            dinvl = pool.tile([P, LPC], F32)
            nc.sync.dma_start(dinvl[:], dinv_d[:])
            px8 = pool.tile([P, LPC, FW], F8 if not second else BF16, tag="px8")
            nc.sync.dma_start(px8[:], px_d.rearrange("(c p) f -> p c f", p=P))
            pxf = pool.tile([P, LPC, FW], F32, tag="pxf")
            nc.scalar.activation(pxf[:], px8[:], AFT.Copy)   # one-time cast
            if second:
                b2rep = pool.tile([P, HID], F32)
                nc.sync.dma_start(b2rep[:], b2_d[:].to_broadcast([P, HID]))
            outsb = pool.tile([P, LPC, FW], F8 if not second else BF16,
                              tag="outsb")

            def body(_=None):
                j_cur = -1
                ps = None
                lv_left = 0

                def finish_chunk(j, ps):
                    t = tb.tile([P, FW], F32, tag="t")
                    nc.vector.scalar_tensor_tensor(
                        out=t[:], in0=ps[:], scalar=dinvl[:, j:j + 1],
                        in1=pxf[:, j, :], op0=MULT, op1=ADD)
                    if not second:
                        nc.scalar.activation(outsb[:, j, :], t[:], AFT.Copy)
                    else:
                        tv = t[:].rearrange("p (m o) -> p m o", o=HID)
                        nc.vector.tensor_tensor(
                            out=tv, in0=tv,
                            in1=b2rep[:, None, :].to_broadcast([P, M, HID]),
                            op=ADD)
                        nc.vector.tensor_relu(out=outsb[:, j, :], in_=t[:])

                # per-chunk level counts to set start/stop flags
                lv_total = {}
                for (j, c0, nidx, lv) in tables["ngath"]:
                    lv_total[j] = lv_total.get(j, 0) + lv

                for gi, (j, c0, nidx, lv) in enumerate(tables["ngath"]):
                    if j != j_cur:
                        if j_cur >= 0:
                            finish_chunk(j_cur, ps)
                        j_cur = j
                        ps = psA.tile([P, FW], F32, tag="psA")
                        lv_left = lv_total[j]
                    G = gbuf.tile([P, NLEV, FW], F8, tag="G")
                    g = G[:, : nidx // P, :]
                    nc.gpsimd.dma_gather(g, src_d[:],
                                         idx_sb[:, c0:c0 + nidx // 16],
                                         nidx, nidx, FW, single_packet=True,
                                         queue_num=gi % 2)
                    for l in range(lv):
                        st = lv_left == lv_total[j]
                        sp = lv_left == 1
                        for hf in (0, 1):
                            nc.tensor.matmul(
                                ps[:, hf * 512:(hf + 1) * 512], idf8[:],
                                g[:, l, hf * 512:(hf + 1) * 512],
                                start=st, stop=sp)
                        lv_left -= 1
                finish_chunk(j_cur, ps)
                nc.sync.dma_start(out_d.rearrange("(c p) f -> p c f", p=P),
                                  outsb[:])

            if reps == 1:
                body()
            else:
                with tc.For_i(0, reps, 1):
                    body()
    nc.compile()
    return nc


def _build_gru(reps=1):
    """GRU over T steps + head.  [h|x] stacked on 128 partitions so each
    chunk needs 2 matmuls (rz fused via contraction; nx|nh in one psum);
    gates via Tanh activations read straight from PSUM (sigmoid as scaled
    tanh -- no act-table swaps)."""
    nc = bacc.Bacc(None, target_bir_lowering=False)
    ROWS = B * NP // NCORES  # 2560
    gx_d = nc.dram_tensor("gx", [T, HID, ROWS], BF16, kind="ExternalInput")
    wrz_d = nc.dram_tensor("wrz", [P, P], BF16, kind="ExternalInput")
    wn_d = nc.dram_tensor("wn", [P, P], BF16, kind="ExternalInput")
    brz_d = nc.dram_tensor("brz", [P, 1], F32, kind="ExternalInput")   # /2
    bnx_d = nc.dram_tensor("bnx", [HID, 1], F32, kind="ExternalInput")
    bnh_d = nc.dram_tensor("bnh", [HID, 1], F32, kind="ExternalInput")
    whead_d = nc.dram_tensor("whead", [HID, 1], F32, kind="ExternalInput")
    bhead_d = nc.dram_tensor("bhead", [1, 1], F32, kind="ExternalInput")
    y_d = nc.dram_tensor("y", [1, ROWS], F32, kind="ExternalOutput")

    CH = 512
    TANH = AFT.Tanh
    SUB = mybir.AluOpType.subtract
    with tile.TileContext(nc) as tc:
        with (
            tc.tile_pool(name="pool", bufs=1) as pool,
            tc.tile_pool(name="ps", bufs=3, space="PSUM") as ps,
            tc.tile_pool(name="sb", bufs=3) as sb,
        ):
            hx = pool.tile([P, T, ROWS], BF16)
            nc.vector.memset(hx[:], 0.0)
            nc.sync.dma_start(hx[HID:P, :, :], gx_d.rearrange("t f n -> f t n"))
            wrz = pool.tile([P, P], BF16)
            nc.sync.dma_start(wrz[:], wrz_d[:])
            wn = pool.tile([P, P], BF16)
            nc.sync.dma_start(wn[:], wn_d[:])
            brz = pool.tile([P, 1], F32)
            nc.sync.dma_start(brz[:], brz_d[:])
            bnx = pool.tile([HID, 1], F32)
            nc.sync.dma_start(bnx[:], bnx_d[:])
            bnh = pool.tile([HID, 1], F32)
            nc.sync.dma_start(bnh[:], bnh_d[:])
            whead = pool.tile([HID, 1], F32)
            nc.sync.dma_start(whead[:], whead_d[:])
            bhead = pool.tile([1, 1], F32)
            nc.sync.dma_start(bhead[:], bhead_d[:])

            def body(_=None):
                h = pool.tile([HID, ROWS], F32, tag="h")
                nc.vector.memset(h[:], 0.0)
                for t in range(T):
                    nc.vector.tensor_copy(out=hx[0:HID, t, :], in_=h[:])
                    for c0 in range(0, ROWS, CH):
                        hxs = hx[:, t, c0:c0 + CH]
                        prz = ps.tile([P, CH], F32, tag="prz")
                        pn = ps.tile([P, CH], F32, tag="pn")
                        nc.tensor.matmul(prz[:], wrz[:], hxs, start=True, stop=True)
                        nc.tensor.matmul(pn[:], wn[:], hxs, start=True, stop=True)
                        # rz' = tanh(0.5*(u + b)); r = 0.5 rz'+0.5, z likewise
                        rz = sb.tile([P, CH], F32, tag="rz")
                        nc.scalar.activation(rz[:], prz[:], TANH,
                                             bias=brz[:], scale=0.5)
                        r = sb.tile([HID, CH], F32, tag="r")
                        nc.vector.tensor_scalar(
                            out=r[:], in0=rz[0:HID, :], scalar1=0.5, scalar2=0.5,
                            op0=MULT, op1=ADD)
                        z = sb.tile([HID, CH], F32, tag="z")
                        nc.vector.tensor_scalar(
                            out=z[:], in0=rz[HID:P, :], scalar1=0.5, scalar2=0.5,
                            op0=MULT, op1=ADD)
                        # n = tanh(nx + bnx + r*(nh + bnh))
                        t1 = sb.tile([HID, CH], F32, tag="t1")
                        nc.vector.scalar_tensor_tensor(
                            out=t1[:], in0=pn[HID:P, :], scalar=bnh[:], in1=r[:],
                            op0=ADD, op1=MULT)
                        nc.vector.scalar_tensor_tensor(
                            out=t1[:], in0=pn[0:HID, :], scalar=bnx[:], in1=t1[:],
                            op0=ADD, op1=ADD)
                        nn = sb.tile([HID, CH], F32, tag="nn")
                        nc.scalar.activation(nn[:], t1[:], TANH)
                        # h' = nn + z*(h - nn)
                        hs = h[:, c0:c0 + CH]
                        nc.vector.tensor_tensor(out=t1[:], in0=hs, in1=nn[:], op=SUB)
                        nc.vector.tensor_tensor(out=t1[:], in0=z[:], in1=t1[:], op=MULT)
                        nc.vector.tensor_tensor(out=hs, in0=nn[:], in1=t1[:], op=ADD)

                ysb = pool.tile([1, ROWS], F32, tag="ysb")
                for c0 in range(0, ROWS, CH):
                    py_full = ps.tile([P, CH], F32, tag="prz")
                    py = py_full[:1, :]
                    nc.tensor.matmul(py[:], whead[:], h[:, c0:c0 + CH], start=True, stop=True)
                    nc.vector.tensor_scalar_add(ysb[:, c0:c0 + CH], py[:], bhead[:])
                nc.sync.dma_start(y_d[:], ysb[:])

            if reps == 1:
                body()
            else:
                with tc.For_i(0, reps, 1):
                    body()
    nc.compile()
    return nc


# ----------------------------------------------------------------------------
# top-level kernel
# ----------------------------------------------------------------------------

def _get_programs(edge_src, edge_dst):
    reps = TIMING_REPS
    h = (hash(edge_src.tobytes()), hash(edge_dst.tobytes()), reps)
    if h not in _CACHE:
        tables = _CACHE.get(("tables", h[0], h[1]))
        if tables is None:
            tables = _prep(edge_src, edge_dst)
            _CACHE[("tables", h[0], h[1])] = tables
        _CACHE[h] = dict(
            tables=tables,
            A0=_build_hop_node(tables, True, reps),
            AS=_build_as(tables, reps),
            H1=_build_hop_full(tables, False, reps),
            H2=_build_hop_full(tables, True, reps),
            GB=_build_gru(reps),
        )
    return _CACHE[h]


def _run(progs, name, in_maps, cores):
    import time
    t0 = time.time()
    r = run_bass_kernel_spmd(progs[name], in_maps, cores)
    LAST_WALLS[name] = time.time() - t0
    return r.results


def kernel(x, edge_src, edge_dst, edge_val,
           W_sp1, b_sp1, W_sp2, b_sp2,
           W_ih, W_hh, b_ih, b_hh, W_head, b_head):
    x = np.asarray(x, np.float32)
    edge_src = np.asarray(edge_src, np.int32)
    edge_dst = np.asarray(edge_dst, np.int32)
    W_sp1 = np.asarray(W_sp1, np.float32)
    b_sp1 = np.asarray(b_sp1, np.float32)
    W_sp2 = np.asarray(W_sp2, np.float32)
    b_sp2 = np.asarray(b_sp2, np.float32)
    W_ih = np.asarray(W_ih, np.float32)
    W_hh = np.asarray(W_hh, np.float32)
    b_ih = np.asarray(b_ih, np.float32)
    b_hh = np.asarray(b_hh, np.float32)
    W_head = np.asarray(W_head, np.float32)
    b_head = np.asarray(b_head, np.float32)

    progs = _get_programs(edge_src, edge_dst)
    tables = progs["tables"]
    order = tables["order"]
    dinv_n = tables["dinv_n"]
    node_of = tables["node_of"]
    cores = list(range(NCORES))

    # x in new-id space, [NP, 16]
    x16 = np.zeros((NP, M), np.float32)
    x16[:N] = x.transpose(1, 0, 2).reshape(N, M)[order]

    dinvw = _wrap(dinv_n)
    dl = [dinv_n[node_of[c]] for c in cores]
    dinvl = [np.ascontiguousarray(d.reshape(LPC, P).T) for d in dl]
    dinv2l = [np.ascontiguousarray((d * d).reshape(LPC, P).T) for d in dl]

    # ---- launch A0: hop1 ----
    in0 = [{"x16": x16, "idxn": tables["nidx"][c],
            "dinvl": dinvl[c], "dinv2l": dinv2l[c], "dinvw": dinvw}
           for c in cores]
    r0 = _run(progs, "A0", in0, cores)
    z1t = np.zeros((NP, 16), np.float32)
    z1s = np.zeros((NP, 64), np.float32)
    for c in cores:
        z1t[node_of[c]] = r0[c]["z1t_s"]
        z1s[node_of[c]] = r0[c]["z1s_s"]

    # ---- launch AS: hop2 + Hin + P0/P1'/W ----
    W1 = W_sp1[:, 0, :]                      # [3, HID]
    w2all = np.zeros((P, 384), np.float32)   # rows (m2,f); cols (m2,k,fo)
    for m2 in range(2):
        for k in range(3):
            w2all[m2 * 64:(m2 + 1) * 64,
                  m2 * 192 + k * 64:m2 * 192 + (k + 1) * 64] = W_sp2[k]
    w2all = w2all.astype(NPBF16)
    ins = [{"idxn": tables["nidx"][c], "z1s": z1s,
            "zx": np.ascontiguousarray(x16[node_of[c]]),
            "z1": np.ascontiguousarray(z1t[node_of[c]]),
            "dinvl": dinvl[c], "dinvsc": dinvl[c] * np.float32(SC),
            "w1": W1, "b1": b_sp1[None, :],
            "w2all": w2all}
           for c in cores]
    rs = _run(progs, "AS", ins, cores)
    Wf = np.zeros((NP, FW), NPF8)
    for c in cores:
        Wf[node_of[c]] = np.asarray(rs[c]["w"])
    Wf[N:] = 0
    idf8 = np.eye(P, dtype=np.float32).astype(NPF8)

    # ---- launch H1: hop3 -> V ----
    inh1 = [{"idxn": tables["nidx"][c], "src": Wf, "idf8": idf8,
             "px": np.asarray(rs[c]["p1"]),
             "dinvl": dinv2l[c]}
            for c in cores]
    rh1 = _run(progs, "H1", inh1, cores)
    Vf = np.zeros((NP, FW), NPF8)
    for c in cores:
        Vf[node_of[c]] = np.asarray(rh1[c]["v"])
    Vf[N:] = 0

    # ---- launch H2: hop4 -> h2 ----
    inh2 = [{"idxn": tables["nidx"][c], "src": Vf, "idf8": idf8,
             "px": np.asarray(rs[c]["p0"]),
             "dinvl": dinvl[c] / np.float32(SC), "b2": b_sp2[None, :]}
            for c in cores]
    rh2 = _run(progs, "H2", inh2, cores)
    H2 = np.zeros((NP, M, HID), NPBF16)
    for c in cores:
        H2[node_of[c]] = np.asarray(rh2[c]["h2"]).reshape(LROWS, M, HID)

    # ---- launch B: GRU + head ----
    ROWS = B * NP // NCORES
    # sequences feature-major: seq[t, f, b*NP + rank]
    seq = H2.reshape(NP, B, T, HID).transpose(2, 3, 1, 0).reshape(T, HID, B * NP)
    wihT = W_ih.T.astype(np.float32)         # [64, 192]
    whhT = W_hh.T.astype(np.float32)
    wrz = np.zeros((P, P), np.float32)       # [h|x] contract -> [r|z]
    wrz[0:64, 0:64] = whhT[:, 0:64]
    wrz[64:128, 0:64] = wihT[:, 0:64]
    wrz[0:64, 64:128] = whhT[:, 64:128]
    wrz[64:128, 64:128] = wihT[:, 64:128]
    wn = np.zeros((P, P), np.float32)        # -> [nx|nh]
    wn[64:128, 0:64] = wihT[:, 128:192]
    wn[0:64, 64:128] = whhT[:, 128:192]
    brz = (np.concatenate([b_ih[0:64] + b_hh[0:64],
                           b_ih[64:128] + b_hh[64:128]]) * 0.5
           ).astype(np.float32)[:, None]     # [128, 1], pre-halved
    inb = []
    for c in cores:
        inb.append({
            "gx": np.ascontiguousarray(seq[:, :, c * ROWS:(c + 1) * ROWS]).astype(NPBF16),
            "wrz": wrz.astype(NPBF16), "wn": wn.astype(NPBF16),
            "brz": brz,
            "bnx": b_ih[128:192].astype(np.float32)[:, None],
            "bnh": b_hh[128:192].astype(np.float32)[:, None],
            "whead": W_head, "bhead": b_head[None, :],
        })
    rb = _run(progs, "GB", inb, cores)
    y_new = np.concatenate([rb[c]["y"][0] for c in cores]).reshape(B, NP)
    y = y_new[:, tables["perm"]]
    return np.ascontiguousarray(y.astype(np.float32))
